# revision 13
# baseline (speedup 1.0000x reference)
"""EGCL (E(n)-equivariant graph conv layer) Trainium2 kernel, 8-core SPMD.

Strategy:
  - Never read the dense [N, NN] `reduce` incidence matrix: it is exactly
    one-hot(edges[:,0]); one-hot tiles are rebuilt on-chip from indices.
  - Sort edges by source node on the host and shard by node range
    (256 nodes/core).  Every core then owns a disjoint slice of both
    outputs, so no collectives are needed at all.
  - Per core: per-128-edge indirect-DMA row gathers fetch src/dst node rows
    (edge-major), PE transposes turn the feature block feature-major for the
    layer-1 matmuls (weights stationary).  Layer 2 runs "flipped"
    (activations stationary) so phi_e lands edge-major, where gating
    (tensor_tensor_reduce dot with Wi) and the scatter matmul want it.
    Aggregation is a windowed is_equal one-hot matmul accumulated in PSUM.
    Node MLP runs on the core's 256 nodes; residual adds in fp32.
"""

import numpy as np

N = 2048
NN_FULL = 65536
F = 128
H = 256
NA = 8
EA = 4
NCORES = 8
NPC = N // NCORES  # nodes per core = 256

_BUILD_CACHE = {}


def _round_up(x, m):
    return ((x + m - 1) // m) * m


def _build_program(E, m0_hi, m1_lo, m1_hi, use_b2e, use_b2x, use_bx3, bi_f, bx3_f):
    """Build + finalize the SPMD Bass program for one core (shared by all 8).

    E: per-core edge capacity (multiple of 512).
    m0_hi / m1_lo..m1_hi: k-tile ranges feeding scatter output M-tiles 0/1.
    """
    import concourse.bacc as bacc
    import concourse.bass as bass
    import concourse.tile as tile
    from concourse import mybir
    from concourse.masks import make_identity

    f16 = mybir.dt.float16
    f32 = mybir.dt.float32
    i32 = mybir.dt.int32
    AF = mybir.ActivationFunctionType
    OP = mybir.AluOpType

    T = E // 128  # edge tiles
    C = E // 512  # chunks

    nc = bacc.Bacc("TRN2", target_bir_lowering=False, debug=False,
                   num_devices=NCORES)

    def din(name, shape, dt=f16):
        return nc.dram_tensor(name, shape, dt, kind="ExternalInput")

    # ---- DRAM I/O -------------------------------------------------------
    node_S = din("node_S", [N, 256])          # [feat(128)|x,y,z,|x|^2,1,pad]
    node_D = din("node_D", [N, 256])          # [feat(128)|x,y,z,1,|x|^2,pad]
    gsidx_d = din("gsidx", [128, T], i32)     # src node id of edge (p, t)
    gdidx_d = din("gdidx", [128, T], i32)
    slocal_d = din("slocal", [128, T], f32)   # core-local src id (or 384 pad)
    eaEM_d = din("eaEM", [E, 4])              # edge_attr edge-major
    w1k0_d = din("w1k0", [128, 512])
    w1k1_d = din("w1k1", [128, 512])
    w1k2_d = din("w1k2", [9, 512])
    b1_d = din("b1", [128, 4], f32)
    w2e_d = din("w2e", [256, 256])
    w2x_d = din("w2x", [256, 256])
    b2e_d = din("b2e", [1, 256])
    b2x_d = din("b2x", [1, 256])
    wib_d = din("wib", [128, 256])
    wx3b_d = din("wx3b", [128, 256])
    n1k0_d = din("n1k0", [128, 256])
    n1k1_d = din("n1k1", [128, 256])
    n1k2_d = din("n1k2", [128, 256])
    n1k3_d = din("n1k3", [8, 256])
    bn1_d = din("bn1", [128, 2], f32)
    n2k0_d = din("n2k0", [128, 128])
    n2k1_d = din("n2k1", [128, 128])
    ftres_d = din("ftres", [128, 256], f32)   # features slice .T + bn2
    ftn_d = din("ftn", [128, 256])            # features slice .T (fp16)
    natT_d = din("natT", [8, 256])
    cres_d = din("cres", [256, 3], f32)

    fout_d = nc.dram_tensor("fout", [128, 256], f32, kind="ExternalOutput")
    cout_d = nc.dram_tensor("cout", [256, 3], f32, kind="ExternalOutput")

    IOA = bass.IndirectOffsetOnAxis

    with tile.TileContext(nc) as tc:
        with tc.tile_pool(name="const", bufs=1) as cp:
            # ---- persistent SBUF tiles ---------------------------------
            msgs = cp.tile([128, T * 260], f16, tag="msgs")
            diff_em = cp.tile([128, T * 3], f16, tag="diff_em")
            gsidx = cp.tile([128, T], i32, tag="gsidx")
            gdidx = cp.tile([128, T], i32, tag="gdidx")
            slocal = cp.tile([128, T], f32, tag="slocal")
            iota = cp.tile([128, 256], i32, tag="iota")
            ident = cp.tile([128, 128], f16, tag="ident")
            w1k0 = cp.tile([128, 512], f16, tag="w1k0")
            w1k1 = cp.tile([128, 512], f16, tag="w1k1")
            w1k2 = cp.tile([9, 512], f16, tag="w1k2")
            b1 = cp.tile([128, 4], f32, tag="b1")
            w2e = cp.tile([128, 2 * 256], f16, tag="w2e")
            w2x = cp.tile([128, 2 * 256], f16, tag="w2x")
            b2e = cp.tile([1, 256], f16, tag="b2e")
            b2x = cp.tile([1, 256], f16, tag="b2x")
            ones1 = cp.tile([1, 128], f16, tag="ones1")
            wib = cp.tile([128, 256], f16, tag="wib")
            wx3b = cp.tile([128, 256], f16, tag="wx3b")
            n1k = cp.tile([128, 3 * 256], f16, tag="n1k")
            n1k3 = cp.tile([8, 256], f16, tag="n1k3")
            bn1 = cp.tile([128, 2], f32, tag="bn1")
            n2k = cp.tile([128, 2 * 128], f16, tag="n2k")
            ftres = cp.tile([128, 256], f32, tag="ftres")
            ftn = cp.tile([128, 256], f16, tag="ftn")
            natT = cp.tile([8, 256], f16, tag="natT")
            cres = cp.tile([128, 6], f32, tag="cres")
            gate_pre = cp.tile([128, T], f32, tag="gate_pre")
            gate_s = cp.tile([128, T], f32, tag="gate_s")
            phix = cp.tile([128, T], f32, tag="phix")
            d2b = cp.tile([128, T], f32, tag="d2b")
            dtmp = cp.tile([128, T], f32, tag="dtmp")
            rinv = cp.tile([128, T], f32, tag="rinv")
            wbuf = cp.tile([128, T], f32, tag="wbuf")
            agg_sb = cp.tile([128, 2 * 260], f16, tag="agg_sb")
            aggT = cp.tile([128, 2 * 256], f16, tag="aggT")
            a_n = cp.tile([128, 2 * 256], f16, tag="a_n")
            fout_sb = cp.tile([128, 256], f32, tag="fout_sb")
            cout_sb = cp.tile([128, 6], f32, tag="cout_sb")

            # ---- load constants ----------------------------------------
            for dst, src in (
                (gsidx, gsidx_d), (gdidx, gdidx_d), (slocal, slocal_d),
                (w1k0, w1k0_d), (w1k1, w1k1_d), (w1k2, w1k2_d), (b1, b1_d),
                (b2e, b2e_d), (b2x, b2x_d), (wib, wib_d), (wx3b, wx3b_d),
                (n1k3, n1k3_d), (bn1, bn1_d), (ftres, ftres_d), (ftn, ftn_d),
                (natT, natT_d),
            ):
                nc.sync.dma_start(out=dst[:], in_=src[:])
            w2e_v = w2e[:].rearrange("p (k n) -> p k n", k=2)
            w2x_v = w2x[:].rearrange("p (k n) -> p k n", k=2)
            n1k_v = n1k[:].rearrange("p (k n) -> p k n", k=3)
            n2k_v = n2k[:].rearrange("p (k n) -> p k n", k=2)
            for k in range(2):
                nc.sync.dma_start(out=w2e_v[:, k, :], in_=w2e_d[k * 128:(k + 1) * 128, :])
                nc.sync.dma_start(out=w2x_v[:, k, :], in_=w2x_d[k * 128:(k + 1) * 128, :])
                nc.sync.dma_start(out=n2k_v[:, k, :],
                                  in_=n2k0_d[:] if k == 0 else n2k1_d[:])
            for k, src in enumerate((n1k0_d, n1k1_d, n1k2_d)):
                nc.sync.dma_start(out=n1k_v[:, k, :], in_=src[:])
            cres_v = cres[:].rearrange("p (m c) -> p m c", m=2)
            for m in range(2):
                nc.sync.dma_start(out=cres_v[:, m, :], in_=cres_d[m * 128:(m + 1) * 128, :])
            nc.gpsimd.memset(ones1[:], 1.0)
            nc.gpsimd.iota(iota[:], pattern=[[1, 256]], base=0, channel_multiplier=0)
            make_identity(nc, ident[:])

            msgs_v = msgs[:].rearrange("p (t c) -> p t c", c=260)
            dem_v = diff_em[:].rearrange("p (t c) -> p t c", c=3)
            AX = mybir.AxisListType
            wib_b = wib[:].rearrange("p (o n) -> p o n", o=1).broadcast_to([128, 4, 256])
            wx3b_b = wx3b[:].rearrange("p (o n) -> p o n", o=1).broadcast_to([128, 4, 256])

            # ---- phase 1: gather + edge MLPs ---------------------------
            with tc.tile_pool(name="l1ps", bufs=3, space="PSUM") as l1ps, \
                 tc.tile_pool(name="l2ps", bufs=3, space="PSUM") as l2ps, \
                 tc.tile_pool(name="tpps", bufs=2, space="PSUM") as tpps, \
                 tc.tile_pool(name="work", bufs=3) as wp, \
                 tc.tile_pool(name="gat", bufs=6) as gp, \
                 tc.tile_pool(name="a1p", bufs=8) as a1p, \
                 tc.tile_pool(name="rhs", bufs=3) as rp:
                for c in range(C):
                    ch = slice(c * 512, (c + 1) * 512)
                    hsT = rp.tile([128, 512], f16, tag="hsT")
                    hdT = rp.tile([128, 512], f16, tag="hdT")
                    t3 = rp.tile([9, 512], f16, tag="t3")
                    a2xc = rp.tile([128, 4 * 256], f16, tag="a2xc")
                    a2xc_v = a2xc[:].rearrange("p (t n) -> p t n", t=4)
                    for j in range(4):
                        t = c * 4 + j
                        jsl = slice(j * 128, (j + 1) * 128)
                        sem = gp.tile([128, 256], f16, tag="sem", name=f"sem{t}")
                        dem = gp.tile([128, 256], f16, tag="dem", name=f"dem{t}")
                        nc.gpsimd.indirect_dma_start(
                            out=sem[:], out_offset=None, in_=node_S[:],
                            in_offset=IOA(ap=gsidx[:, t:t + 1], axis=0))
                        nc.gpsimd.indirect_dma_start(
                            out=dem[:], out_offset=None, in_=node_D[:],
                            in_offset=IOA(ap=gdidx[:, t:t + 1], axis=0))
                        # feature blocks -> feature-major via PE transpose
                        tpa = tpps.tile([128, 128], f16, tag="tp")
                        nc.tensor.transpose(out=tpa[:], in_=sem[:, 0:128],
                                            identity=ident[:])
                        nc.vector.tensor_copy(hsT[:, jsl], tpa[:])
                        tpb = tpps.tile([128, 128], f16, tag="tp")
                        nc.tensor.transpose(out=tpb[:], in_=dem[:, 0:128],
                                            identity=ident[:])
                        nc.scalar.copy(hdT[:, jsl], tpb[:])
                        # misc row block: [xs*xd(3), |xs|^2, |xd|^2, ea(4)]
                        prod = wp.tile([128, 9], f16, tag="prod")
                        nc.vector.tensor_tensor(out=prod[:, 0:5],
                                                in0=sem[:, 128:133],
                                                in1=dem[:, 128:133], op=OP.mult)
                        nc.sync.dma_start(out=prod[:, 5:9],
                                          in_=eaEM_d[t * 128:(t + 1) * 128, :])
                        tpc = tpps.tile([128, 128], f16, tag="tp")
                        nc.tensor.transpose(out=tpc[0:9, :], in_=prod[:],
                                            identity=ident[:])
                        nc.vector.tensor_copy(t3[:, jsl], tpc[0:9, :])
                        # diff (edge-major) + d^2
                        nc.vector.tensor_tensor(out=dem_v[:, t, :],
                                                in0=sem[:, 128:131],
                                                in1=dem[:, 128:131],
                                                op=OP.subtract)
                    a1_tiles = []
                    for m in range(4):
                        msl = slice(m * 128, (m + 1) * 128)
                        ps = l1ps.tile([128, 512], f32, tag="l1")
                        nc.tensor.matmul(out=ps[:], lhsT=w1k0[:, msl],
                                         rhs=hsT[:], start=True, stop=False)
                        nc.tensor.matmul(out=ps[:], lhsT=w1k1[:, msl],
                                         rhs=hdT[:], start=False, stop=False)
                        nc.tensor.matmul(out=ps[:], lhsT=w1k2[:, msl],
                                         rhs=t3[:], start=False, stop=True)
                        a1 = a1p.tile([128, 512], f16, tag="a1")
                        nc.scalar.activation(out=a1[:], in_=ps[:], func=AF.Silu,
                                             bias=b1[:, m:m + 1], scale=1.0)
                        a1_tiles.append(a1)
                    for j in range(4):
                        t = c * 4 + j
                        jsl = slice(j * 128, (j + 1) * 128)
                        # edge-MLP layer 2, flipped: activations stationary
                        ps2 = l2ps.tile([128, 256], f32, tag="l2")
                        nc.tensor.matmul(out=ps2[:], lhsT=a1_tiles[0][:, jsl],
                                         rhs=w2e_v[:, 0, :], start=True, stop=False)
                        nc.tensor.matmul(out=ps2[:], lhsT=a1_tiles[1][:, jsl],
                                         rhs=w2e_v[:, 1, :], start=False,
                                         stop=not use_b2e)
                        if use_b2e:
                            nc.tensor.matmul(out=ps2[:], lhsT=ones1[:],
                                             rhs=b2e[:], start=False, stop=True)
                        nc.scalar.activation(out=msgs_v[:, t, 0:256], in_=ps2[:],
                                             func=AF.Silu)
                        # coord-MLP layer 2
                        ps2x = l2ps.tile([128, 256], f32, tag="l2")
                        nc.tensor.matmul(out=ps2x[:], lhsT=a1_tiles[2][:, jsl],
                                         rhs=w2x_v[:, 0, :], start=True, stop=False)
                        nc.tensor.matmul(out=ps2x[:], lhsT=a1_tiles[3][:, jsl],
                                         rhs=w2x_v[:, 1, :], start=False,
                                         stop=not use_b2x)
                        if use_b2x:
                            nc.tensor.matmul(out=ps2x[:], lhsT=ones1[:],
                                             rhs=b2x[:], start=False, stop=True)
                        nc.scalar.activation(out=a2xc_v[:, j, :], in_=ps2x[:],
                                             func=AF.Silu)


                    c4 = slice(c * 4, (c + 1) * 4)
                    scrg = wp.tile([128, 4 * 256], f16, tag="scrg")
                    scrg_v = scrg[:].rearrange("p (t n) -> p t n", t=4)
                    nc.gpsimd.tensor_tensor(out=scrg_v, in0=msgs_v[:, c4, 0:256],
                                            in1=wib_b, op=OP.mult)
                    nc.vector.reduce_sum(
                        out=gate_pre[:, c4].rearrange("p (t o) -> p t o", o=1),
                        in_=scrg_v, axis=AX.X)
                    scrx = wp.tile([128, 4 * 256], f16, tag="scrg")
                    scrx_v = scrx[:].rearrange("p (t n) -> p t n", t=4)
                    nc.gpsimd.tensor_tensor(out=scrx_v, in0=a2xc_v,
                                            in1=wx3b_b, op=OP.mult)
                    nc.vector.reduce_sum(
                        out=phix[:, c4].rearrange("p (t o) -> p t o", o=1),
                        in_=scrx_v, axis=AX.X)
                    scr3 = wp.tile([128, 12], f16, tag="scr3")
                    scr3_v = scr3[:].rearrange("p (t n) -> p t n", t=4)
                    nc.vector.tensor_tensor(out=scr3_v, in0=dem_v[:, c4, :],
                                            in1=dem_v[:, c4, :], op=OP.mult)
                    nc.vector.reduce_sum(
                        out=d2b[:, c4].rearrange("p (t o) -> p t o", o=1),
                        in_=scr3_v, axis=AX.X)

            # ---- phase 1.5: batched per-edge scalar chain --------------
            nc.scalar.activation(out=gate_s[:], in_=gate_pre[:],
                                 func=AF.Sigmoid, bias=bi_f)
            nc.scalar.activation(out=dtmp[:], in_=d2b[:], func=AF.Sqrt)
            nc.scalar.activation(out=dtmp[:], in_=dtmp[:], func=AF.Copy,
                                 bias=1.0, scale=1.0)
            nc.vector.reciprocal(rinv[:], dtmp[:])
            if use_bx3:
                nc.scalar.activation(out=phix[:], in_=phix[:], func=AF.Copy,
                                     bias=bx3_f, scale=1.0)
            nc.vector.tensor_tensor(out=wbuf[:], in0=phix[:], in1=rinv[:],
                                    op=OP.mult)
            for t in range(T):
                nc.vector.tensor_scalar(
                    out=msgs_v[:, t, 0:256], in0=msgs_v[:, t, 0:256],
                    scalar1=gate_s[:, t:t + 1], scalar2=None, op0=OP.mult)
                nc.vector.tensor_scalar(
                    out=msgs_v[:, t, 256:259], in0=dem_v[:, t, :],
                    scalar1=wbuf[:, t:t + 1], scalar2=None, op0=OP.mult)

            # ---- phase 2: scatter-by-source + node MLP -----------------
            m_ks = (list(range(0, m0_hi)), list(range(m1_lo, m1_hi)))
            with tc.tile_pool(name="aggps", bufs=2, space="PSUM") as aggps, \
                 tc.tile_pool(name="ndps", bufs=2, space="PSUM") as ndps, \
                 tc.tile_pool(name="tp2ps", bufs=2, space="PSUM") as tp2ps, \
                 tc.tile_pool(name="ohp", bufs=3) as ohp:
                agg_ps = [aggps.tile([128, 260], f32, tag="agg", name=f"agg{i}")
                          for i in range(2)]
                ks_all = sorted(set(m_ks[0]) | set(m_ks[1]))
                for k in ks_all:
                    oh = ohp.tile([128, 256], f16, tag="oh")
                    nc.vector.tensor_scalar(out=oh[:], in0=iota[:],
                                            scalar1=slocal[:, k:k + 1],
                                            scalar2=None, op0=OP.is_equal)
                    for m in range(2):
                        if k not in m_ks[m]:
                            continue
                        nc.tensor.matmul(
                            out=agg_ps[m][:, 0:259],
                            lhsT=oh[:, m * 128:(m + 1) * 128],
                            rhs=msgs_v[:, k, 0:259],
                            start=(k == m_ks[m][0]), stop=(k == m_ks[m][-1]))

                agg_sb_v = agg_sb[:].rearrange("p (m c) -> p m c", m=2)
                cout_v = cout_sb[:].rearrange("p (m c) -> p m c", m=2)
                for m in range(2):
                    nc.vector.tensor_copy(agg_sb_v[:, m, 0:259],
                                          agg_ps[m][:, 0:259])
                    nc.vector.tensor_tensor(out=cout_v[:, m, :],
                                            in0=agg_ps[m][:, 256:259],
                                            in1=cres_v[:, m, :], op=OP.add)
                    nc.sync.dma_start(out=cout_d[m * 128:(m + 1) * 128, :],
                                      in_=cout_v[:, m, :])
                # transpose agg[., 0:256] -> aggT [feat, node]
                aggT_v = aggT[:].rearrange("p (h n) -> p h n", h=2)
                for m in range(2):
                    for h in range(2):
                        tp = tp2ps.tile([128, 128], f16, tag="tp2")
                        nc.tensor.transpose(
                            out=tp[:],
                            in_=agg_sb_v[:, m, h * 128:(h + 1) * 128],
                            identity=ident[:])
                        nc.vector.tensor_copy(
                            aggT_v[:, h, m * 128:(m + 1) * 128], tp[:])
                # node MLP
                a_n_v = a_n[:].rearrange("p (h n) -> p h n", h=2)
                for mt in range(2):
                    msl = slice(mt * 128, (mt + 1) * 128)
                    ps = ndps.tile([128, 256], f32, tag="nd")
                    nc.tensor.matmul(out=ps[:], lhsT=n1k_v[:, 0, msl], rhs=ftn[:],
                                     start=True, stop=False)
                    nc.tensor.matmul(out=ps[:], lhsT=n1k_v[:, 1, msl],
                                     rhs=aggT_v[:, 0, :], start=False, stop=False)
                    nc.tensor.matmul(out=ps[:], lhsT=n1k_v[:, 2, msl],
                                     rhs=aggT_v[:, 1, :], start=False, stop=False)
                    nc.tensor.matmul(out=ps[:], lhsT=n1k3[:, msl], rhs=natT[:],
                                     start=False, stop=True)
                    nc.scalar.activation(out=a_n_v[:, mt, :], in_=ps[:],
                                         func=AF.Silu, bias=bn1[:, mt:mt + 1])
                ps = ndps.tile([128, 256], f32, tag="nd")
                nc.tensor.matmul(out=ps[:], lhsT=n2k_v[:, 0, :], rhs=a_n_v[:, 0, :],
                                 start=True, stop=False)
                nc.tensor.matmul(out=ps[:], lhsT=n2k_v[:, 1, :], rhs=a_n_v[:, 1, :],
                                 start=False, stop=True)
                nc.vector.tensor_tensor(out=fout_sb[:], in0=ps[:], in1=ftres[:],
                                        op=OP.add)
                nc.sync.dma_start(out=fout_d[:], in_=fout_sb[:])

    nc.finalize()
    return nc


def _prep_inputs(coords, features, edges, node_attr, edge_attr,
                 We1, be1, We2, be2, Wx1, bx1, Wx2, bx2, Wx3, bx3,
                 Wn1, bn1, Wn2, bn2, Wi, bi):
    """Host-side sharding + weight packing."""
    f16 = np.float16
    f32 = np.float32

    coords = np.asarray(coords, f32)
    features = np.asarray(features, f32)
    node_attr = np.asarray(node_attr, f32)
    edge_attr = np.asarray(edge_attr, f32)
    src = np.asarray(edges)[:, 0].astype(np.int64)
    dst = np.asarray(edges)[:, 1].astype(np.int64)

    order = np.argsort(src, kind="stable")
    s_s, d_s, ea_s = src[order], dst[order], edge_attr[order]
    bounds = np.searchsorted(s_s, np.arange(0, N + 1, NPC))
    counts = np.diff(bounds)
    E = max(512, _round_up(int(counts.max()), 512))
    T = E // 128

    # node tables (fp16), shared across cores
    normsq = (coords ** 2).sum(1)
    node_S = np.zeros((N, 256), f16)
    node_S[:, :F] = features
    node_S[:, 128:131] = coords
    node_S[:, 131] = normsq
    node_S[:, 132] = 1.0
    node_D = node_S.copy()
    node_D[:, 131] = 1.0
    node_D[:, 132] = normsq

    w_e, w_x = We1[256], Wx1[256]
    w1k0 = np.concatenate([We1[0:128], Wx1[0:128]], 1).astype(f16)
    w1k1 = np.concatenate([We1[128:256], Wx1[128:256]], 1).astype(f16)
    w1k2 = np.concatenate([
        np.stack([-2 * w_e, -2 * w_e, -2 * w_e, w_e, w_e,
                  We1[257], We1[258], We1[259], We1[260]]),
        np.stack([-2 * w_x, -2 * w_x, -2 * w_x, w_x, w_x,
                  Wx1[257], Wx1[258], Wx1[259], Wx1[260]]),
    ], 1).astype(f16)
    b1 = np.stack([be1[:128], be1[128:], bx1[:128], bx1[128:]], 1).astype(f32)
    shared = dict(
        node_S=node_S, node_D=node_D,
        w1k0=w1k0, w1k1=w1k1, w1k2=w1k2, b1=b1,
        w2e=We2.astype(f16), w2x=Wx2.astype(f16),
        b2e=be2[None, :].astype(f16), b2x=bx2[None, :].astype(f16),
        wib=np.tile(Wi[:, 0], (128, 1)).astype(f16),
        wx3b=np.tile(Wx3[:, 0], (128, 1)).astype(f16),
        n1k0=Wn1[0:128].astype(f16), n1k1=Wn1[128:256].astype(f16),
        n1k2=Wn1[256:384].astype(f16), n1k3=Wn1[384:392].astype(f16),
        bn1=np.stack([bn1[:128], bn1[128:]], 1).astype(f32),
        n2k0=Wn2[0:128].astype(f16), n2k1=Wn2[128:256].astype(f16),
    )

    in_maps = []
    m0_hi_u, m1_lo_u, m1_hi_u = 1, T, 1
    for c in range(NCORES):
        sl = slice(int(bounds[c]), int(bounds[c + 1]))
        n_c = int(counts[c])
        gsrc = np.zeros(E, np.int64)
        gdst = np.zeros(E, np.int64)
        eac = np.zeros((E, EA), f32)
        slocal = np.full(E, 384, np.int64)
        gsrc[:n_c] = s_s[sl]
        gdst[:n_c] = d_s[sl]
        eac[:n_c] = ea_s[sl]
        slocal[:n_c] = s_s[sl] - NPC * c

        # k-tile ranges for the two output M-tiles (union across cores)
        b_c = int(np.searchsorted(slocal[:n_c], 128))
        kt_real = max(1, -(-n_c // 128))
        m0_hi_c = max(1, -(-b_c // 128)) if b_c > 0 else 0
        m0_hi_u = max(m0_hi_u, m0_hi_c)
        m1_lo_u = min(m1_lo_u, b_c // 128 if b_c < n_c else max(kt_real - 1, 0))
        m1_hi_u = max(m1_hi_u, kt_real)

        nsl = slice(NPC * c, NPC * (c + 1))
        core = dict(shared)
        core.update(
            gsidx=gsrc.reshape(T, 128).T.astype(np.int32).copy(),
            gdidx=gdst.reshape(T, 128).T.astype(np.int32).copy(),
            slocal=slocal.reshape(T, 128).T.astype(np.float32).copy(),
            eaEM=eac.astype(f16),
            ftres=(features[nsl].T + bn2[:, None]).astype(f32).copy(),
            ftn=features[nsl].T.astype(f16).copy(),
            natT=node_attr[nsl].T.astype(f16).copy(),
            cres=coords[nsl].astype(f32).copy(),
        )
        in_maps.append(core)

    m1_lo_u = min(m1_lo_u, m1_hi_u - 1)
    key = (E, int(m0_hi_u), int(m1_lo_u), int(m1_hi_u),
           bool(np.any(be2)), bool(np.any(bx2)), bool(np.any(bx3)),
           float(bi[0]), float(bx3[0]))
    return key, in_maps


def _run(key, in_maps, trace=False):
    from concourse import bass_utils
    if key not in _BUILD_CACHE:
        _BUILD_CACHE[key] = _build_program(*key)
    nc = _BUILD_CACHE[key]
    return bass_utils.run_bass_kernel_spmd(
        nc, in_maps, list(range(NCORES)), trace=trace)


def kernel(coords, features, edges, reduce, node_attr, edge_attr,
           We1, be1, We2, be2, Wx1, bx1, Wx2, bx2, Wx3, bx3,
           Wn1, bn1, Wn2, bn2, Wi, bi, _trace=False):
    args = [np.asarray(a) for a in
            (coords, features, edges, node_attr, edge_attr, We1, be1, We2, be2,
             Wx1, bx1, Wx2, bx2, Wx3, bx3, Wn1, bn1, Wn2, bn2, Wi, bi)]
    key, in_maps = _prep_inputs(*args)
    res = _run(key, in_maps, trace=_trace)
    features_out = np.concatenate(
        [res.results[c]["fout"].T for c in range(NCORES)], 0)
    coords_out = np.concatenate(
        [res.results[c]["cout"] for c in range(NCORES)], 0)
    if _trace:
        kernel._last_result = res
    return (coords_out.astype(np.float32), features_out.astype(np.float32))
